# revision 1
# baseline (speedup 1.0000x reference)
"""GRUNetMultiLabel kernel for 8 Trainium2 NeuronCores.

Strategy (per sharding_hint): data-parallel over batch across the 8
cores — GRU/fc/embedding weights replicated, x and hidden state sharded
on B (64 -> 8 per core). Each core: embedding gather, the big
input-projection GEMM (xe @ W_ih.T, precomputed for all T), the
sequential GRU recurrence over T=256, the FC head, sigmoid and 0.5
thresholding. Full inputs in, full outputs out.

Self-contained: hardcodes shapes; no sibling imports.
"""
import numpy as np

B, T, V, D, H, O = 64, 256, 50000, 512, 1024, 64
N_CORES = 8

_COMPILED = {}


def _get_fn():
    if "f" in _COMPILED:
        return _COMPILED["f"]
    import jax
    import jax.numpy as jnp

    def per_core(x_c, emb, W_ih, W_hh, b_ih, b_hh, W_fc, b_fc):
        xe = emb[x_c]                                        # [b, T, D]
        gates_x = jnp.einsum("btd,gd->btg", xe, W_ih) + b_ih  # [b, T, 3H]

        def step(h, gx):
            gh = h @ W_hh.T + b_hh
            xr, xz, xn = jnp.split(gx, 3, axis=-1)
            hr, hz, hn = jnp.split(gh, 3, axis=-1)
            r = jax.nn.sigmoid(xr + hr)
            z = jax.nn.sigmoid(xz + hz)
            n = jnp.tanh(xn + r * hn)
            h_new = (1.0 - z) * n + z * h
            return h_new, h_new

        h0 = jnp.zeros((x_c.shape[0], H), dtype=xe.dtype)
        _, hs = jax.lax.scan(step, h0, jnp.swapaxes(gates_x, 0, 1))
        out = jnp.swapaxes(hs, 0, 1)                         # [b, T, H]
        logits = jnp.einsum("bth,oh->bto", out, W_fc) + b_fc
        proba = jax.nn.sigmoid(logits)
        labels = (proba > 0.5).astype(jnp.float32)
        return proba, labels

    f = jax.pmap(per_core,
                 in_axes=(0, None, None, None, None, None, None, None),
                 devices=jax.devices()[:N_CORES])
    _COMPILED["f"] = f
    return f


def kernel(x, emb, W_ih, W_hh, b_ih, b_hh, W_fc, b_fc):
    x = np.asarray(x)
    in_dtype = x.dtype
    xs = np.ascontiguousarray(x.astype(np.int32).reshape(N_CORES, B // N_CORES, T))
    emb = np.asarray(emb, np.float32)
    W_ih = np.asarray(W_ih, np.float32)
    W_hh = np.asarray(W_hh, np.float32)
    b_ih = np.asarray(b_ih, np.float32)
    b_hh = np.asarray(b_hh, np.float32)
    W_fc = np.asarray(W_fc, np.float32)
    b_fc = np.asarray(b_fc, np.float32)

    f = _get_fn()
    proba, labels = f(xs, emb, W_ih, W_hh, b_ih, b_hh, W_fc, b_fc)
    proba = np.asarray(proba, np.float32).reshape(B, T, O)
    labels = np.asarray(labels, np.float32).reshape(B, T, O)
    del in_dtype
    return proba, labels

